# revision 3
# baseline (speedup 1.0000x reference)
"""LoRA-GRU Trainium2 kernel.

Strategy
--------
The GRU x-side projections (xo = x @ Wx^T + LoRA) do not depend on the
recurrent state, only the h-side does.  Batch rows are grouped by their
LoRA agent and each of the 8 cores processes one agent's rows, so the
per-agent LoRA folds into a dense merged weight on the host:
    W_hat = W + alpha * B[g] @ A[g]
leaving a plain dense GRU on the device.

Device layout is "transposed": hidden dim on SBUF partitions, batch rows
on the free dim.  State h^T is stored folded as [128, 4*C] where
tile[p, k*C + j] = h[row j, 128*k + p].  The recurrent matmuls run in
out^T form (merged weights as the stationary operand in fp16, which gets
fast-weight-load), producing gate pre-activations directly in the folded
layout, so the sigmoid/tanh gating runs on all 128 ACT/DVE lanes.

The x-side projections are computed just-in-time in blocks of GS steps
(moving dim GS*C) and the per-step xo for the r/z gates is accumulated
into the gate PSUM via an identity matmul; biases are folded into xo at
block-copy time via a per-partition tensor_scalar add.
"""

import os
import sys
import math
import numpy as np

for _p in ("/opt/trn_rl_repo",):
    if _p not in sys.path and os.path.isdir(_p):
        sys.path.insert(0, _p)

from contextlib import ExitStack

import concourse.bass as bass
import concourse.bacc as bacc
import concourse.tile as tile
import concourse.mybir as mybir
from concourse.bass_utils import run_bass_kernel_spmd

dt = mybir.dt
AF = mybir.ActivationFunctionType

# Problem constants (nn_LoRA_GRU_67817533604217)
T, B, IN, H, R, NA = 128, 256, 512, 512, 16, 8
ALPHA = 1.0
H3 = 3 * H
KC = IN // 128          # 4 contraction chunks
NG = H3 // 128          # 12 output row chunks (m): 0-3 r, 4-7 z, 8-11 n
NCORES = 8


def _build_program(C: int, GS: int):
    """Build the SPMD bass program for per-core row capacity C and
    x-projection block size GS steps."""
    assert T % GS == 0
    NB = T // GS
    NX = GS * C                      # moving dim of x-side matmuls
    assert NX <= 512 and 4 * C <= 512

    nc = bacc.Bacc("TRN2", target_bir_lowering=False, debug=False,
                   num_devices=NCORES)

    xT = nc.dram_tensor("xT", [KC, 128, T * C], dt.float16,
                        kind="ExternalInput").ap()
    wxT = nc.dram_tensor("wxT", [KC, 128, H3], dt.float16,
                         kind="ExternalInput").ap()
    whT = nc.dram_tensor("whT", [KC, 128, H3], dt.float16,
                         kind="ExternalInput").ap()
    h0 = nc.dram_tensor("h0", [128, KC * C], dt.float16,
                        kind="ExternalInput").ap()
    ident = nc.dram_tensor("ident", [128, 128], dt.float16,
                           kind="ExternalInput").ap()
    biasx = nc.dram_tensor("biasx", [128, NG], dt.float32,
                           kind="ExternalInput").ap()
    out = nc.dram_tensor("out", [T, 128, 4 * C], dt.float16,
                         kind="ExternalOutput").ap()

    with tile.TileContext(nc) as tc, ExitStack() as ctx:
        wpool = ctx.enter_context(tc.tile_pool(name="weights", bufs=1))
        xpool = ctx.enter_context(tc.tile_pool(name="xin", bufs=3 * KC))
        slabp = ctx.enter_context(tc.tile_pool(name="slab", bufs=3))
        hpool = ctx.enter_context(tc.tile_pool(name="state", bufs=3))
        gpool = ctx.enter_context(tc.tile_pool(name="gates", bufs=2))
        psg = ctx.enter_context(tc.tile_pool(name="psg", bufs=2, space="PSUM"))
        psx = ctx.enter_context(tc.tile_pool(name="psx", bufs=2, space="PSUM"))

        # ---- resident weights / constants ----
        wh_sb, wx_sb = [], []
        for k in range(KC):
            wt = wpool.tile([128, H3], dt.float16, tag=f"whsb{k}")
            nc.sync.dma_start(wt[:], whT[k])
            wh_sb.append(wt)
            xt_ = wpool.tile([128, H3], dt.float16, tag=f"wxsb{k}")
            nc.sync.dma_start(xt_[:], wxT[k])
            wx_sb.append(xt_)
        id_sb = wpool.tile([128, 128], dt.float16, tag="id")
        nc.sync.dma_start(id_sb[:], ident)
        bx_sb = wpool.tile([128, NG], dt.float32, tag="bx")
        nc.sync.dma_start(bx_sb[:], biasx)

        hT = hpool.tile([128, KC * C], dt.float16)
        nc.sync.dma_start(hT[:], h0)

        # ---- x-side projection blocks ----
        def emit_xblock_dma(b):
            xin = []
            for k in range(KC):
                t_ = xpool.tile([128, NX], dt.float16, tag="xin")
                nc.sync.dma_start(t_[:], xT[k][:, b * NX:(b + 1) * NX])
                xin.append(t_)
            slab = slabp.tile([128, GS * NG * C], dt.float16, tag="slab")
            return xin, slab

        def emit_xblock_m(xin, slab, m):
            # one output row-chunk m of block b: psum = sum_k WxT_k[m] @ x
            ps = psx.tile([128, NX], dt.float32, tag="psx")
            for k in range(KC):
                nc.tensor.matmul(ps[:], wx_sb[k][:, m * 128:(m + 1) * 128],
                                 xin[k][:], start=(k == 0), stop=(k == KC - 1))
            # copy psum -> slab slices (per step-within-block), folding bias
            dst = slab[:].rearrange("p (t m j) -> p t m j",
                                    t=GS, m=NG, j=C)[:, :, m, :]
            src = ps[:].rearrange("p (t j) -> p t j", t=GS)
            nc.vector.tensor_scalar_add(dst, src, bx_sb[:, m:m + 1])

        # ---- one GRU step ----
        def emit_step(t, slab, hT_cur):
            tt = t % GS
            sl = slab[:]
            base = tt * NG * C
            xo_rz_r = sl[:, base:base + 4 * C]
            xo_rz_z = sl[:, base + 4 * C:base + 8 * C]
            xo_n = sl[:, base + 8 * C:base + 12 * C]

            ps_r = psg.tile([128, 4 * C], dt.float32, tag="ps_r")
            ps_z = psg.tile([128, 4 * C], dt.float32, tag="ps_z")
            ps_n = psg.tile([128, 4 * C], dt.float32, tag="ps_n")

            # identity-fold xo into r/z psums (start of their accum groups)
            nc.tensor.matmul(ps_r[:], id_sb[:], xo_rz_r, start=True,
                             stop=False, skip_group_check=True)

            # r-group recurrent matmuls (m = 0..3)
            for ml in range(4):
                for k in range(KC):
                    nc.tensor.matmul(
                        ps_r[:, ml * C:(ml + 1) * C],
                        wh_sb[k][:, ml * 128:(ml + 1) * 128],
                        hT_cur[:, k * C:(k + 1) * C],
                        start=False, stop=(k == KC - 1),
                        skip_group_check=True)
            r_t = gpool.tile([128, 4 * C], dt.float32, tag="r")
            nc.scalar.activation(r_t[:], ps_r[:], AF.Sigmoid)

            # n-group (m = 8..11)
            for ml in range(4):
                m = 8 + ml
                for k in range(KC):
                    nc.tensor.matmul(
                        ps_n[:, ml * C:(ml + 1) * C],
                        wh_sb[k][:, m * 128:(m + 1) * 128],
                        hT_cur[:, k * C:(k + 1) * C],
                        start=(k == 0), stop=(k == KC - 1),
                        skip_group_check=True)
            nt = gpool.tile([128, 4 * C], dt.float32, tag="nt")
            nc.vector.tensor_mul(nt[:], r_t[:], ps_n[:])
            ns = gpool.tile([128, 4 * C], dt.float32, tag="ns")
            nc.vector.tensor_add(ns[:], nt[:], xo_n)
            n_t = gpool.tile([128, 4 * C], dt.float32, tag="n")
            nc.scalar.activation(n_t[:], ns[:], AF.Tanh)

            # z-group (m = 4..7)
            nc.tensor.matmul(ps_z[:], id_sb[:], xo_rz_z, start=True,
                             stop=False, skip_group_check=True)
            for ml in range(4):
                m = 4 + ml
                for k in range(KC):
                    nc.tensor.matmul(
                        ps_z[:, ml * C:(ml + 1) * C],
                        wh_sb[k][:, m * 128:(m + 1) * 128],
                        hT_cur[:, k * C:(k + 1) * C],
                        start=False, stop=(k == KC - 1),
                        skip_group_check=True)
            z_t = gpool.tile([128, 4 * C], dt.float32, tag="z")
            nc.scalar.activation(z_t[:], ps_z[:], AF.Sigmoid)

            # h' = n + z * (h - n)
            d_t = gpool.tile([128, 4 * C], dt.float32, tag="d")
            nc.vector.tensor_sub(d_t[:], hT_cur[:], n_t[:])
            e_t = gpool.tile([128, 4 * C], dt.float32, tag="e")
            nc.vector.tensor_mul(e_t[:], z_t[:], d_t[:])
            hT_new = hpool.tile([128, KC * C], dt.float16)
            nc.vector.tensor_add(hT_new[:], n_t[:], e_t[:])

            nc.sync.dma_start(out[t], hT_new[:])
            return hT_new

        # ---- main schedule: prologue blocks 0,1 then steps with JIT x ----
        blocks = {}
        for b in range(min(2, NB)):
            xin, slab = emit_xblock_dma(b)
            for m in range(NG):
                emit_xblock_m(xin, slab, m)
            blocks[b] = slab

        hT_cur = hT
        pending = None  # (xin, slab, next_m) of block being emitted
        for t in range(T):
            b = t // GS
            tt = t % GS
            nb = b + 2
            if nb < NB:
                if tt == 0:
                    pending = [emit_xblock_dma(nb), 0]
                (xin, slab), m0 = pending
                m1 = ((tt + 1) * NG + GS - 1) // GS
                for m in range(m0, min(m1, NG)):
                    emit_xblock_m(xin, slab, m)
                pending[1] = min(m1, NG)
                if tt == GS - 1:
                    blocks[nb] = slab
                    pending = None
            hT_cur = emit_step(t, blocks[b], hT_cur)
            if tt == GS - 1 and b in blocks:
                del blocks[b]

    nc.compile()
    return nc


_PROGRAM_CACHE = {}
_LAST_RESULTS = None


def _get_program(C, GS):
    key = (C, GS)
    if key not in _PROGRAM_CACHE:
        _PROGRAM_CACHE[key] = _build_program(C, GS)
    return _PROGRAM_CACHE[key]


def _choose_shape(max_count):
    C = max(32, int(math.ceil(max_count / 8.0)) * 8)
    if C > 128:
        raise ValueError(f"agent group too large for this kernel: {max_count}")
    for GS in (8, 4, 2, 1):
        if GS * C <= 512:
            return C, GS
    raise ValueError("no valid GS")


def kernel(x, h, Wx, bx, Ax, Bx, Wh, bh, Ah, Bh, agent_id):
    x = np.asarray(x, dtype=np.float32)
    h = np.asarray(h, dtype=np.float32)
    Wx = np.asarray(Wx, dtype=np.float32)
    bx = np.asarray(bx, dtype=np.float32)
    Ax = np.asarray(Ax, dtype=np.float32)
    Bx = np.asarray(Bx, dtype=np.float32)
    Wh = np.asarray(Wh, dtype=np.float32)
    bh = np.asarray(bh, dtype=np.float32)
    Ah = np.asarray(Ah, dtype=np.float32)
    Bh = np.asarray(Bh, dtype=np.float32)
    agent = np.asarray(agent_id).astype(np.int64).reshape(-1)

    idx = [np.nonzero(agent == g)[0] for g in range(NA)]
    max_count = max(len(i) for i in idx)
    C, GS = _choose_shape(max_count)
    nc = _get_program(C, GS)

    bsum = (bx + bh).astype(np.float32)
    bias_tile = np.ascontiguousarray(bsum.reshape(NG, 128).T)  # [128, NG]
    ident = np.eye(128, dtype=np.float16)

    in_maps = []
    for g in range(NA):
        ig = idx[g]
        n = len(ig)
        xp = np.zeros((T, C, IN), np.float32)
        if n:
            xp[:, :n, :] = x[:, ig, :]
        xTg = np.ascontiguousarray(
            xp.transpose(2, 0, 1).reshape(KC, 128, T * C)).astype(np.float16)

        Wxg = Wx + ALPHA * (Bx[g] @ Ax[g])
        Whg = Wh + ALPHA * (Bh[g] @ Ah[g])
        wxTg = np.ascontiguousarray(
            Wxg.T.reshape(KC, 128, H3)).astype(np.float16)
        whTg = np.ascontiguousarray(
            Whg.T.reshape(KC, 128, H3)).astype(np.float16)

        hp = np.zeros((C, H), np.float32)
        if n:
            hp[:n, :] = h[0, ig, :]
        h0g = np.ascontiguousarray(
            hp.T.reshape(KC, 128, C).transpose(1, 0, 2).reshape(128, KC * C)
        ).astype(np.float16)

        in_maps.append({
            "xT": xTg, "wxT": wxTg, "whT": whTg, "h0": h0g,
            "ident": ident, "biasx": bias_tile,
        })

    res = run_bass_kernel_spmd(nc, in_maps, core_ids=list(range(NCORES)))
    global _LAST_RESULTS
    _LAST_RESULTS = res

    out = np.zeros((T, B, H), np.float32)
    for g in range(NA):
        n = len(idx[g])
        if n == 0:
            continue
        ob = res.results[g]["out"].astype(np.float32)       # [T, 128, 4C]
        obr = ob.reshape(T, 128, KC, C)[:, :, :, :n]        # [T, p, k, j]
        out[:, idx[g], :] = obr.transpose(0, 3, 2, 1).reshape(T, n, H)

    h_last = out[T - 1:T].copy()
    return out, h_last
